# revision 24
# baseline (speedup 1.0000x reference)
"""Multi-head attention + residual + LayerNorm, tensor-parallel over heads
across 8 Trainium2 NeuronCores.

Reference computation (SEQ=2048, BATCH=2, D_MODEL=1024, H=16, D=64):
    qkv = h @ Wqkv.T ; per-(b,head) softmax((q k^T)/8, key-padding mask) @ v
    out = LayerNorm(h + concat_heads @ Wo.T) * gamma + beta

v4 strategy (2 heads per core):
- Projections (QKV, Wo) in fp8e4 DoubleRow; scores in bf16 with the two
  heads' K=64 matmuls row-tiled onto array halves (concurrent); AV as
  fp8 DoubleRow pairs of key tiles with a ones-column producing Z.
- Wqkv/Wo scaled x16 host-side; the resulting x256 on the attention
  branch is matched by hrows x256 and cancelled exactly by LayerNorm
  scale invariance.
- PE density: one shared 3-deep [128,1024] PSUM ring serves scores,
  QKV and Wo; QKV(b1) and Wo matmul groups are interleaved as fillers
  into the attention stream so the PE never idles long enough for the
  HAM clock gate to re-throttle to 1.2 GHz.
- Query panels interleaved mod 4: the vec AllToAll splits into 4
  chunks launched per finished panel; Wo+residual+LN per row block,
  two panels behind its chunk. Z reciprocals are computed on a
  [128,8] partition-packed tile.
"""
import sys

if "/opt/trn_rl_repo" not in sys.path:
    sys.path.insert(0, "/opt/trn_rl_repo")

from collections import deque

import numpy as np
import ml_dtypes

import bass_rust
import concourse.bass as bass
import concourse.mybir as mybir
import concourse.tile as tile
from concourse.bass_utils import run_bass_kernel_spmd

FP8NP = ml_dtypes.float8_e4m3fn
F32 = mybir.dt.float32
BF = mybir.dt.bfloat16
FP8 = mybir.dt.float8e4
DR = mybir.MatmulPerfMode.DoubleRow

SEQ, BATCH, DM = 2048, 2, 1024
NH, DH = 16, 64
NC_ = 8                      # cores
HPC = NH // NC_              # heads per core (2)
ROWS = SEQ * BATCH // NC_    # flat rows per core (512)
NT = SEQ // 128              # key tiles (16)
NP = 4                       # query panels (s mod 4 interleaved)
LN_EPS = 1e-5
NEG = -1e30
WS = 16.0                    # fp8 weight scale (power of two)
EXPB = -2.0                  # exp bias (cancels in U/Z), guards fp8 overflow


# ---------------------------------------------------------------------------
# walrus compat: this toolchain accepts at most ONE sync-wait per instruction.
# Split excess waits (and >1 updates on non-DMA instructions) onto adjacent
# same-engine NOPs after Tile scheduling.
# ---------------------------------------------------------------------------
_DMAISH = ("InstDMACopy", "InstDMATranspose", "DmaTranspose", "InstCollectiveCompute")


def _legalize_waits(nc: bass.Bass) -> int:
    n = 0
    for fn in nc.m.functions:
        for bb in fn.blocks:
            insts = bb.instructions
            i = 0
            while i < len(insts):
                inst = insts[i]
                si = inst.sync_info
                if si is None:
                    i += 1
                    continue
                waits = list(si.on_wait)
                updates = list(si.on_update)
                is_dma = any(k in type(inst).__name__ for k in _DMAISH)
                split_upd = (not is_dma) and len(updates) > 1
                if len(waits) <= 1 and not split_upd:
                    i += 1
                    continue
                keep_u = updates if not split_upd else updates[:1]
                extra_u = [] if not split_upd else updates[1:]
                eng = nc.engines[inst.engine]
                before = []
                for w in waits[1:]:
                    nop = eng.nop(nofuse=True).ins
                    _pop_last(nc, nop)
                    nop.sync_info = bass_rust.SyncInfo(on_wait=[w], on_update=[])
                    before.append(nop)
                after = []
                for u in extra_u:
                    nop = eng.nop(nofuse=True).ins
                    _pop_last(nc, nop)
                    nop.sync_info = bass_rust.SyncInfo(on_wait=[], on_update=[u])
                    after.append(nop)
                inst.sync_info = bass_rust.SyncInfo(on_wait=waits[:1], on_update=keep_u)
                insts[i:i + 1] = before + [inst] + after
                n += len(before) + len(after)
                i += len(before) + 1 + len(after)
    return n


def _pop_last(nc, inst):
    for fn in nc.m.functions:
        for bb in fn.blocks:
            lst = bb.instructions
            if lst and lst[-1] is inst:
                lst.pop()
                return
    for fn in nc.m.functions:
        for bb in fn.blocks:
            lst = bb.instructions
            for k in range(len(lst) - 1, -1, -1):
                if lst[k] is inst:
                    del lst[k]
                    return
    raise RuntimeError("fresh nop not found")


# ---------------------------------------------------------------------------
# kernel graph
# ---------------------------------------------------------------------------
def _build(masked_full, any_mixed):
    """masked_full: set of (t, b) key-tiles fully masked (skipped entirely).
    any_mixed: whether partially-masked tiles exist (bias from mb applied)."""
    nc = bass.Bass()

    hT8 = nc.declare_dram_parameter("hT8", [BATCH, DM, SEQ], FP8, isOutput=False)
    wT = nc.declare_dram_parameter("wT", [DM, 3 * HPC * DH], FP8, isOutput=False)
    woT = nc.declare_dram_parameter("woT", [DM, DM], FP8, isOutput=False)
    hrows = nc.declare_dram_parameter("hrows", [ROWS, DM], F32, isOutput=False)
    gamma = nc.declare_dram_parameter("gamma", [DM], F32, isOutput=False)
    beta = nc.declare_dram_parameter("beta", [DM], F32, isOutput=False)
    if any_mixed:
        mb = nc.declare_dram_parameter("mb", [128, NT * BATCH], F32, isOutput=False)
    out = nc.declare_dram_parameter("out", [ROWS, DM], F32, isOutput=True)

    a2a_in = nc.dram_tensor("a2a_in", [NP, NC_, 128, 128], FP8)
    a2a_out = nc.dram_tensor("a2a_out", [NP, NC_, 128, 128], FP8)
    warm_in = nc.dram_tensor("warm_in", [NC_, 64], FP8)
    warm_out = nc.dram_tensor("warm_out", [NC_, 64], FP8)
    zb = nc.dram_tensor("zb", [BATCH, HPC, NP, 512], F32)
    zr = nc.dram_tensor("zr", [BATCH, HPC, NP, 512], F32)

    # per batch: unmasked key tiles, grouped into consecutive DoubleRow pairs
    unmasked = {b: [t for t in range(NT) if (t, b) not in masked_full] for b in range(BATCH)}
    groups = {}
    for b in range(BATCH):
        um = unmasked[b]
        g, i = [], 0
        while i < len(um):
            if i + 1 < len(um) and um[i + 1] == um[i] + 1:
                g.append((um[i], um[i] + 1))
                i += 2
            else:
                g.append((um[i],))
                i += 1
        groups[b] = g

    with tile.TileContext(nc) as tc:
        with tc.tile_pool(name="big", bufs=1) as big, \
             tc.tile_pool(name="epool", bufs=3) as epool, \
             tc.tile_pool(name="small", bufs=4) as small, \
             tc.tile_pool(name="ps_pool", bufs=3, space="PSUM") as ps_pool, \
             tc.tile_pool(name="u_ps", bufs=1, space="PSUM") as u_ps:

            # ---- persistent SBUF tensors ----
            wt_sb = big.tile([128, 8, 3 * HPC * DH], FP8)     # Wqkv shard^T chunks
            wot_sb = big.tile([128, 8, DM], FP8)              # Wo^T chunks (full)
            qT = big.tile([128, BATCH, SEQ], BF)
            kT = big.tile([128, BATCH, SEQ], BF)
            vT = big.tile([128, BATCH, SEQ], BF)
            v_sb = big.tile([128, BATCH, NT, 160], FP8)       # [v_h0|1|pad|v_h1|1|pad]
            vecT = big.tile([128, NP, NC_, 128], FP8)         # chunked vec^T
            eps_sb = big.tile([128, 1], F32)
            expb_sb = big.tile([128, 1], F32)
            gam_sb = big.tile([128, DM], F32)
            bet_sb = big.tile([128, DM], F32)
            if any_mixed:
                mb_sb = big.tile([128, NT * BATCH], F32)

            # warm the collective pipeline: tiny AllToAll, first in the gpsimd
            # queue (fires as soon as the init barrier ends)
            wz = small.tile([128, 4], FP8, name="wz", tag="wz")
            nc.vector.memset(wz, 0.0)
            nc.gpsimd.dma_start(out=warm_in[:, :], in_=wz)
            nc.gpsimd.collective_compute(
                "AllToAll", mybir.AluOpType.bypass,
                replica_groups=[list(range(NC_))],
                ins=[warm_in[:]], outs=[warm_out[:]],
            )

            nc.vector.memset(expb_sb, EXPB)
            nc.vector.memset(eps_sb, LN_EPS * (WS * WS) ** 2)
            nc.vector.memset(v_sb[:, :, :, 64:65], 1.0)
            nc.vector.memset(v_sb[:, :, :, 144:145], 1.0)

            # weights for first QKV pair, then h(b=0), then the rest
            wT_v = wT.rearrange("(m p) c -> m p c", p=128)     # [8, 128, 384]
            woT_v = woT.rearrange("(m p) c -> m p c", p=128)
            nc.gpsimd.dma_start(out=wt_sb[:, 0:2, :], in_=wT_v[0:2].rearrange("s p c -> p s c"))
            hts = []
            for b in range(BATCH):
                htb = big.tile([128, 8, SEQ], FP8, name=f"hts{b}", tag="hts", bufs=2)
                hts.append(htb)
            hT8_v = hT8.rearrange("b (m p) c -> b m p c", p=128)
            for mm in range(4):
                nc.gpsimd.dma_start(
                    out=hts[0][:, 2 * mm:2 * mm + 2, :],
                    in_=hT8_v[0, 2 * mm:2 * mm + 2].rearrange("s p c -> p s c"))
            for mm in range(1, 4):
                nc.gpsimd.dma_start(
                    out=wt_sb[:, 2 * mm:2 * mm + 2, :],
                    in_=wT_v[2 * mm:2 * mm + 2].rearrange("s p c -> p s c"))
            for mm in range(4):
                nc.gpsimd.dma_start(
                    out=hts[1][:, 2 * mm:2 * mm + 2, :],
                    in_=hT8_v[1, 2 * mm:2 * mm + 2].rearrange("s p c -> p s c"))
            if any_mixed:
                nc.gpsimd.dma_start(out=mb_sb, in_=mb[:, :])
            for mm in range(4):
                nc.gpsimd.dma_start(
                    out=wot_sb[:, 2 * mm:2 * mm + 2, :],
                    in_=woT_v[2 * mm:2 * mm + 2].rearrange("s p c -> p s c"))
            nc.gpsimd.dma_start(out=gam_sb, in_=bass.AP(tensor=gamma, offset=0, ap=[[0, 128], [1, DM]]))
            nc.gpsimd.dma_start(out=bet_sb, in_=bass.AP(tensor=beta, offset=0, ap=[[0, 128], [1, DM]]))

            CH = HPC * DH  # 128 channels per section

            def qk_group(b, ct, dest, p):
                """One QKV projection group: 4 DR matmuls + copy to bf16."""
                htb = hts[b]
                ps = ps_pool.tile([128, 1024], F32, name="ps", tag="ps")
                for mm in range(4):
                    nc.tensor.matmul(
                        ps[:, 0:512],
                        lhsT=wt_sb[:, 2 * mm:2 * mm + 2, ct * CH:(ct + 1) * CH],
                        rhs=htb[:, 2 * mm:2 * mm + 2, p * 512:(p + 1) * 512],
                        start=(mm == 0), stop=(mm == 3), perf_mode=DR,
                    )
                nc.vector.tensor_copy(dest[:, b, p * 512:(p + 1) * 512], ps[:, 0:512])

            def v_mm(b, p):
                """v projection panel into vT (bf16)."""
                htb = hts[b]
                ps = ps_pool.tile([128, 1024], F32, name="ps", tag="ps")
                for mm in range(4):
                    nc.tensor.matmul(
                        ps[:, 0:512],
                        lhsT=wt_sb[:, 2 * mm:2 * mm + 2, 2 * CH:3 * CH],
                        rhs=htb[:, 2 * mm:2 * mm + 2, p * 512:(p + 1) * 512],
                        start=(mm == 0), stop=(mm == 3), perf_mode=DR,
                    )
                nc.vector.tensor_copy(vT[:, b, p * 512:(p + 1) * 512], ps[:, 0:512])

            def v_tc(b):
                """All transposes + fp8 casts for batch b, after all v_mm(b,*):
                keeps the vector queue free of transpose-gated entries ahead of
                later vT copies."""
                for p in range(4):
                    for h in range(HPC):
                        vpk = small.tile([128, 4, DH], BF, name="vpk", tag="vpk", bufs=4)
                        nc.sync.dma_start(
                            out=vpk, in_=vT[h * DH:(h + 1) * DH, b, p * 512:(p + 1) * 512],
                            transpose=True)
                        nc.vector.tensor_copy(
                            v_sb[:, b, p * 4:p * 4 + 4, :].rearrange(
                                "c t (h d) -> c t h d", h=2)[:, :, h, 0:DH],
                            vpk)

            # filler queue: independent matmul groups woven into the attention
            # stream to keep the PE dense (HAM stays at K=8/8)
            fillers = deque()

            def emit_fillers(n):
                for _ in range(n):
                    if not fillers:
                        return
                    fillers.popleft()()

            qT4 = qT.rearrange("c b (q f) -> c b q f", f=4)
            vec5 = vecT.rearrange("c p j (k two) -> c p j k two", two=2)

            def attn_panel(b, p, fps=1):
                ups = [u_ps.tile([65, 512], F32, name=f"ups{h}", tag=f"ups{h}")
                       for h in range(HPC)]
                grp = groups[b]
                for gi, g in enumerate(grp):
                    e2t = epool.tile([128, 2, 1024], FP8, name="e2", tag="e2", bufs=3)
                    for slot, t in enumerate(g):
                        sc = ps_pool.tile([128, 1024], F32, name="sc", tag="ps")
                        for h in range(HPC):
                            nc.tensor.matmul(
                                sc[:, h * 512:(h + 1) * 512],
                                lhsT=kT[h * DH:(h + 1) * DH, b, t * 128:(t + 1) * 128],
                                rhs=qT4[h * DH:(h + 1) * DH, b, :, p],
                                start=True, stop=True,
                            )
                        if any_mixed:
                            bias = mb_sb[:, t * BATCH + b:t * BATCH + b + 1]
                        else:
                            bias = expb_sb[:, 0:1]
                        nc.scalar.activation(
                            out=e2t[:, slot, :], in_=sc,
                            func=mybir.ActivationFunctionType.Exp,
                            bias=bias, scale=1.0 / (np.sqrt(DH) * WS * WS),
                        )
                    first, last = (gi == 0), (gi == len(grp) - 1)
                    for h in range(HPC):
                        if len(g) == 2:
                            nc.tensor.matmul(
                                ups[h],
                                lhsT=v_sb[:, b, g[0]:g[0] + 2, :].rearrange(
                                    "c t (h d) -> c t h d", h=2)[:, :, h, 0:DH + 1],
                                rhs=e2t[:, :, h * 512:(h + 1) * 512],
                                start=first, stop=last, perf_mode=DR,
                                skip_group_check=True,
                            )
                        else:
                            nc.tensor.matmul(
                                ups[h],
                                lhsT=v_sb[:, b, g[0], 80 * h:80 * h + DH + 1],
                                rhs=e2t[:, 0, h * 512:(h + 1) * 512],
                                start=first, stop=last,
                                skip_group_check=True,
                            )
                    emit_fillers(fps)
                # Z rows -> DRAM, packed reciprocal, broadcast back, divide
                u_sbs = []
                for h in range(HPC):
                    u_sb = small.tile([65, 512], F32, name="u_sb", tag=f"u_sb{h}", bufs=2)
                    nc.vector.tensor_copy(u_sb, ups[h])
                    nc.gpsimd.dma_start(out=zb[b, h, p, :], in_=u_sb[64:65, :])
                    u_sbs.append(u_sb)
                zpk = small.tile([128, 8], F32, name="zpk", tag="zpk", bufs=2)
                nc.gpsimd.dma_start(
                    out=zpk,
                    in_=bass.AP(tensor=zb, offset=(b * HPC * NP + p) * 512,
                                ap=[[NP * 512, 2], [1, 512]]))
                nc.vector.reciprocal(zpk, zpk)
                nc.gpsimd.dma_start(
                    out=bass.AP(tensor=zr, offset=(b * HPC * NP + p) * 512,
                                ap=[[NP * 512, 2], [1, 512]]),
                    in_=zpk)
                for h in range(HPC):
                    zrep = small.tile([64, 512], F32, name="zrep", tag="zrep", bufs=2)
                    nc.gpsimd.dma_start(
                        out=zrep,
                        in_=bass.AP(tensor=zr, offset=((b * HPC + h) * NP + p) * 512,
                                    ap=[[0, 64], [1, 512]]))
                    nc.vector.tensor_mul(
                        out=vec5[h * DH:(h + 1) * DH, p, :, :, b],
                        in0=u_sbs[h][0:DH, :], in1=zrep,
                    )

            def a2a_chunk(p):
                nc.gpsimd.dma_start(
                    out=a2a_in[p].rearrange("j c r -> c j r"),
                    in_=vecT[:, p, :, :])
                nc.gpsimd.collective_compute(
                    "AllToAll", mybir.AluOpType.bypass,
                    replica_groups=[list(range(NC_))],
                    ins=[a2a_in[p, :]], outs=[a2a_out[p, :]],
                )

            # Wo/LN per row block; DMAs early, matmul groups as fillers later
            wo_state = {}

            def wo_dma(p):
                vt = small.tile([128, 8, 128], FP8, name="vt", tag="vt", bufs=2)
                nc.gpsimd.dma_start(out=vt, in_=a2a_out[p].rearrange("i c r -> c i r"))
                hr = big.tile([128, DM], F32, name="hr", tag="hr", bufs=2)
                nc.gpsimd.dma_start(
                    out=hr,
                    in_=bass.AP(tensor=hrows, offset=2 * p * DM,
                                ap=[[8 * DM, 64], [DM, 2], [1, DM]]))
                x = big.tile([128, DM], F32, name="x", tag="x", bufs=2)
                wo_state[p] = (vt, hr, x)

            def wo_mh(p, mh):
                vt, hr, x = wo_state[p]
                ps = ps_pool.tile([128, 1024], F32, name="ps", tag="ps")
                for c4 in range(4):
                    nc.tensor.matmul(
                        ps[:, 0:512],
                        lhsT=vt[:, 2 * c4:2 * c4 + 2, :],
                        rhs=wot_sb[:, 2 * c4:2 * c4 + 2, mh * 512:(mh + 1) * 512],
                        start=(c4 == 0), stop=(c4 == 3), perf_mode=DR,
                    )
                nc.vector.tensor_add(
                    out=x[:, mh * 512:(mh + 1) * 512],
                    in0=ps[:, 0:512], in1=hr[:, mh * 512:(mh + 1) * 512],
                )

            def wo_ln(p):
                _, _, x = wo_state.pop(p)
                stats = small.tile([128, 2, 6], F32, name="stats", tag="stats")
                nc.vector.bn_stats(out=stats[:, 0, :], in_=x[:, 0:512])
                nc.vector.bn_stats(out=stats[:, 1, :], in_=x[:, 512:1024])
                mv = small.tile([128, 2], F32, name="mv", tag="mv")
                nc.vector.bn_aggr(out=mv, in_=stats)
                rstd = small.tile([128, 1], F32, name="rstd", tag="rstd")
                nc.scalar.activation(out=rstd, in_=mv[:, 1:2],
                                     func=mybir.ActivationFunctionType.Sqrt,
                                     bias=eps_sb, scale=1.0)
                nc.vector.reciprocal(rstd, rstd)
                nc.vector.tensor_scalar(
                    out=x, in0=x,
                    scalar1=mv[:, 0:1], scalar2=rstd,
                    op0=mybir.AluOpType.subtract, op1=mybir.AluOpType.mult,
                )
                nc.vector.tensor_mul(out=x, in0=x, in1=gam_sb)
                nc.vector.tensor_add(out=x, in0=x, in1=bet_sb)
                nc.gpsimd.dma_start(
                    out=bass.AP(tensor=out, offset=2 * p * DM,
                                ap=[[8 * DM, 64], [DM, 2], [1, DM]]),
                    in_=x)

            # ---- emission ----
            # QKV b0 dense (warms the PE), b1 as fillers inside attention
            for ct, dest in ((1, kT), (0, qT)):
                for p in range(4):
                    qk_group(0, ct, dest, p)
            for p in range(4):
                v_mm(0, p)
            v_tc(0)
            for ct, dest in ((1, kT), (0, qT)):
                for p in range(4):
                    fillers.append(lambda b=1, ct=ct, dest=dest, p=p: qk_group(b, ct, dest, p))
            for p in range(4):
                fillers.append(lambda b=1, p=p: v_mm(b, p))
            fillers.append(lambda b=1: v_tc(b))

            for p in range(NP):
                for b in range(BATCH):
                    # all 12 QKV(b1) fillers must land inside (b0, p0) so
                    # (b1, p0)'s AV never precedes its v_group in the PE queue
                    attn_panel(b, p, fps=2 if (p == 0 and b == 0) else 1)
                a2a_chunk(p)
                if p >= 1:
                    wo_dma(p - 1)
                    fillers.append(lambda q=p - 1: wo_mh(q, 0))
                    fillers.append(lambda q=p - 1: wo_mh(q, 1))
                    fillers.append(lambda q=p - 1: wo_ln(q))
            while fillers:
                fillers.popleft()()
            wo_dma(NP - 1)
            wo_mh(NP - 1, 0)
            wo_mh(NP - 1, 1)
            wo_ln(NP - 1)

    _legalize_waits(nc)
    return nc


# ---------------------------------------------------------------------------
# host wrapper
# ---------------------------------------------------------------------------
_CACHE = {}
LAST_RESULT = None


def _get_nc(attn_mask: np.ndarray):
    masked_full = frozenset(
        (t, b) for t in range(NT) for b in range(BATCH)
        if attn_mask[t * 128:(t + 1) * 128, b].all()
    )
    any_mixed = any(
        attn_mask[t * 128:(t + 1) * 128, b].any() and (t, b) not in masked_full
        for t in range(NT) for b in range(BATCH)
    )
    key = (masked_full, any_mixed)
    if key not in _CACHE:
        _CACHE[key] = _build(masked_full, any_mixed)
    return _CACHE[key]


def _in_maps(h, attn_mask, Wqkv, Wo, gamma, beta):
    h = np.asarray(h, np.float32)
    attn_mask = np.asarray(attn_mask, bool)
    Wqkv = np.asarray(Wqkv, np.float32)
    Wo = np.asarray(Wo, np.float32)
    gamma = np.asarray(gamma, np.float32)
    beta = np.asarray(beta, np.float32)

    hT8 = np.ascontiguousarray(h.transpose(1, 2, 0)).astype(FP8NP)   # [B, DM, SEQ]
    h_flat = h.reshape(SEQ * BATCH, DM)
    woT = np.ascontiguousarray(Wo.T * WS).astype(FP8NP)              # [DM(ch), DM(m)]
    mb = np.zeros((128, NT * BATCH), np.float32)
    for t in range(NT):
        for b in range(BATCH):
            mb[:, t * BATCH + b] = np.where(
                attn_mask[t * 128:(t + 1) * 128, b], NEG, 0.0) + EXPB

    maps = []
    for c in range(NC_):
        h0, h1 = HPC * c, HPC * c + 1
        rows = []
        for sec in range(3):  # q, k, v
            for hh in (h0, h1):
                rows.append(Wqkv[sec * NH * DH + hh * DH: sec * NH * DH + (hh + 1) * DH])
        wTc = np.ascontiguousarray(np.concatenate(rows, 0).T * WS).astype(FP8NP)
        m = {
            "hT8": hT8,
            "wT": wTc,
            "woT": woT,
            "hrows": np.ascontiguousarray(h_flat[ROWS * c: ROWS * (c + 1)] * (WS * WS)),
            "gamma": gamma,
            "beta": beta,
            "mb": mb,
        }
        maps.append(m)
    return maps


def kernel(h, attn_mask, Wqkv, Wo, gamma, beta, _trace=False):
    global LAST_RESULT
    mask = np.asarray(attn_mask, bool)
    nc = _get_nc(mask)
    maps = _in_maps(h, attn_mask, Wqkv, Wo, gamma, beta)
    any_mixed = any(
        mask[t * 128:(t + 1) * 128, b].any() and not mask[t * 128:(t + 1) * 128, b].all()
        for t in range(NT) for b in range(BATCH)
    )
    if not any_mixed:
        for m in maps:
            m.pop("mb", None)
    res = run_bass_kernel_spmd(nc, maps, core_ids=list(range(NC_)), trace=_trace)
    LAST_RESULT = res
    full = np.concatenate([res.results[c]["out"] for c in range(NC_)], 0)
    out = full.reshape(SEQ, BATCH, DM)
    if _trace:
        return out, res.exec_time_ns
    return out


# revision 27
# speedup vs baseline: 1.1663x; 1.1663x over previous
"""Multi-head attention + residual + LayerNorm, tensor-parallel over heads
across 8 Trainium2 NeuronCores.

Reference computation (SEQ=2048, BATCH=2, D_MODEL=1024, H=16, D=64):
    qkv = h @ Wqkv.T ; per-(b,head) softmax((q k^T)/8, key-padding mask) @ v
    out = LayerNorm(h + concat_heads @ Wo.T) * gamma + beta

v4 strategy (2 heads per core):
- Projections (QKV, Wo) in fp8e4 DoubleRow; scores in bf16 with the two
  heads' K=64 matmuls row-tiled onto array halves (concurrent); AV as
  fp8 DoubleRow pairs of key tiles with a ones-column producing Z.
- Wqkv/Wo scaled x16 host-side; the resulting x256 on the attention
  branch is matched by hrows x256 and cancelled exactly by LayerNorm
  scale invariance.
- PE density: one shared 3-deep [128,1024] PSUM ring serves scores,
  QKV and Wo; QKV(b1) and Wo matmul groups are interleaved as fillers
  into the attention stream so the PE never idles long enough for the
  HAM clock gate to re-throttle to 1.2 GHz.
- Query panels interleaved mod 4: the vec AllToAll splits into 4
  chunks launched per finished panel; Wo+residual+LN per row block,
  two panels behind its chunk. Z reciprocals are computed on a
  [128,8] partition-packed tile.
"""
import sys

if "/opt/trn_rl_repo" not in sys.path:
    sys.path.insert(0, "/opt/trn_rl_repo")

from collections import deque

import numpy as np
import ml_dtypes

import bass_rust
import concourse.bass as bass
import concourse.mybir as mybir
import concourse.tile as tile
from concourse.bass_utils import run_bass_kernel_spmd

FP8NP = ml_dtypes.float8_e4m3fn
F32 = mybir.dt.float32
BF = mybir.dt.bfloat16
FP8 = mybir.dt.float8e4
DR = mybir.MatmulPerfMode.DoubleRow

SEQ, BATCH, DM = 2048, 2, 1024
NH, DH = 16, 64
NC_ = 8                      # cores
HPC = NH // NC_              # heads per core (2)
ROWS = SEQ * BATCH // NC_    # flat rows per core (512)
NT = SEQ // 128              # key tiles (16)
NP = 4                       # query panels (s mod 4 interleaved)
LN_EPS = 1e-5
NEG = -1e30
WS = 16.0                    # fp8 weight scale (power of two)
EXPB = -2.0                  # exp bias (cancels in U/Z), guards fp8 overflow


# ---------------------------------------------------------------------------
# walrus compat: this toolchain accepts at most ONE sync-wait per instruction.
# Split excess waits (and >1 updates on non-DMA instructions) onto adjacent
# same-engine NOPs after Tile scheduling.
# ---------------------------------------------------------------------------
_DMAISH = ("InstDMACopy", "InstDMATranspose", "DmaTranspose", "InstCollectiveCompute")


def _legalize_waits(nc: bass.Bass) -> int:
    n = 0
    for fn in nc.m.functions:
        for bb in fn.blocks:
            insts = bb.instructions
            i = 0
            while i < len(insts):
                inst = insts[i]
                si = inst.sync_info
                if si is None:
                    i += 1
                    continue
                waits = list(si.on_wait)
                updates = list(si.on_update)
                is_dma = any(k in type(inst).__name__ for k in _DMAISH)
                split_upd = (not is_dma) and len(updates) > 1
                if len(waits) <= 1 and not split_upd:
                    i += 1
                    continue
                keep_u = updates if not split_upd else updates[:1]
                extra_u = [] if not split_upd else updates[1:]
                eng = nc.engines[inst.engine]
                before = []
                for w in waits[1:]:
                    nop = eng.nop(nofuse=True).ins
                    _pop_last(nc, nop)
                    nop.sync_info = bass_rust.SyncInfo(on_wait=[w], on_update=[])
                    before.append(nop)
                after = []
                for u in extra_u:
                    nop = eng.nop(nofuse=True).ins
                    _pop_last(nc, nop)
                    nop.sync_info = bass_rust.SyncInfo(on_wait=[], on_update=[u])
                    after.append(nop)
                inst.sync_info = bass_rust.SyncInfo(on_wait=waits[:1], on_update=keep_u)
                insts[i:i + 1] = before + [inst] + after
                n += len(before) + len(after)
                i += len(before) + 1 + len(after)
    return n


def _pop_last(nc, inst):
    for fn in nc.m.functions:
        for bb in fn.blocks:
            lst = bb.instructions
            if lst and lst[-1] is inst:
                lst.pop()
                return
    for fn in nc.m.functions:
        for bb in fn.blocks:
            lst = bb.instructions
            for k in range(len(lst) - 1, -1, -1):
                if lst[k] is inst:
                    del lst[k]
                    return
    raise RuntimeError("fresh nop not found")


# ---------------------------------------------------------------------------
# kernel graph
# ---------------------------------------------------------------------------
def _build(masked_full, any_mixed):
    """masked_full: set of (t, b) key-tiles fully masked (skipped entirely).
    any_mixed: whether partially-masked tiles exist (bias from mb applied)."""
    nc = bass.Bass()

    hT8 = nc.declare_dram_parameter("hT8", [BATCH, DM, SEQ], FP8, isOutput=False)
    wT = nc.declare_dram_parameter("wT", [DM, 3 * HPC * DH], FP8, isOutput=False)
    woT = nc.declare_dram_parameter("woT", [DM, DM], FP8, isOutput=False)
    hrows = nc.declare_dram_parameter("hrows", [ROWS, DM], F32, isOutput=False)
    gamma = nc.declare_dram_parameter("gamma", [DM], F32, isOutput=False)
    beta = nc.declare_dram_parameter("beta", [DM], F32, isOutput=False)
    if any_mixed:
        mb = nc.declare_dram_parameter("mb", [128, NT * BATCH], F32, isOutput=False)
    out = nc.declare_dram_parameter("out", [ROWS, DM], F32, isOutput=True)

    a2a_in = nc.dram_tensor("a2a_in", [NP, NC_, 128, 128], FP8)
    a2a_out = nc.dram_tensor("a2a_out", [NP, NC_, 128, 128], FP8)
    warm_in = nc.dram_tensor("warm_in", [NC_, 64], FP8)
    warm_out = nc.dram_tensor("warm_out", [NC_, 64], FP8)
    zb = nc.dram_tensor("zb", [BATCH, HPC, NP, 512], F32)
    zr = nc.dram_tensor("zr", [BATCH, HPC, NP, 512], F32)

    # per batch: unmasked key tiles, grouped into consecutive DoubleRow pairs
    unmasked = {b: [t for t in range(NT) if (t, b) not in masked_full] for b in range(BATCH)}
    groups = {}
    for b in range(BATCH):
        um = unmasked[b]
        g, i = [], 0
        while i < len(um):
            if i + 1 < len(um) and um[i + 1] == um[i] + 1:
                g.append((um[i], um[i] + 1))
                i += 2
            else:
                g.append((um[i],))
                i += 1
        groups[b] = g

    with tile.TileContext(nc) as tc:
        with tc.tile_pool(name="big", bufs=1) as big, \
             tc.tile_pool(name="epool", bufs=3) as epool, \
             tc.tile_pool(name="small", bufs=4) as small, \
             tc.tile_pool(name="ps_pool", bufs=3, space="PSUM") as ps_pool, \
             tc.tile_pool(name="u_ps", bufs=1, space="PSUM") as u_ps:

            # ---- persistent SBUF tensors ----
            wt_sb = big.tile([128, 8, 3 * HPC * DH], FP8)     # Wqkv shard^T chunks
            wot_sb = big.tile([128, 8, DM], FP8)              # Wo^T chunks (full)
            qT = big.tile([128, BATCH, SEQ], BF)
            kT = big.tile([128, BATCH, SEQ], BF)
            vT = big.tile([128, BATCH, SEQ], BF)
            v_sb = big.tile([128, BATCH, NT, 160], FP8)       # [v_h0|1|pad|v_h1|1|pad]
            vecT = big.tile([128, NP, NC_, 128], FP8)         # chunked vec^T
            eps_sb = big.tile([128, 1], F32)
            expb_sb = big.tile([128, 1], F32)
            gam_sb = big.tile([128, DM], F32)
            bet_sb = big.tile([128, DM], F32)
            if any_mixed:
                mb_sb = big.tile([128, NT * BATCH], F32)

            zero_sb = big.tile([128, 1], F32)
            nc.vector.memset(zero_sb, 0.0)
            nc.vector.memset(expb_sb, EXPB)
            nc.vector.memset(eps_sb, LN_EPS * (WS * WS) ** 2)
            nc.vector.memset(v_sb[:, :, :, 64:65], 1.0)
            nc.vector.memset(v_sb[:, :, :, 144:145], 1.0)

            # weights for first QKV pair, then h(b=0), then the rest
            wT_v = wT.rearrange("(m p) c -> m p c", p=128)     # [8, 128, 384]
            woT_v = woT.rearrange("(m p) c -> m p c", p=128)
            nc.gpsimd.dma_start(out=wt_sb[:, 0:2, :], in_=wT_v[0:2].rearrange("s p c -> p s c"))
            hts = []
            for b in range(BATCH):
                htb = big.tile([128, 8, SEQ], FP8, name=f"hts{b}", tag="hts", bufs=2)
                hts.append(htb)
            hT8_v = hT8.rearrange("b (m p) c -> b m p c", p=128)
            for mm in range(4):
                nc.gpsimd.dma_start(
                    out=hts[0][:, 2 * mm:2 * mm + 2, :],
                    in_=hT8_v[0, 2 * mm:2 * mm + 2].rearrange("s p c -> p s c"))
            for mm in range(1, 4):
                nc.gpsimd.dma_start(
                    out=wt_sb[:, 2 * mm:2 * mm + 2, :],
                    in_=wT_v[2 * mm:2 * mm + 2].rearrange("s p c -> p s c"))
            for mm in range(4):
                nc.gpsimd.dma_start(
                    out=hts[1][:, 2 * mm:2 * mm + 2, :],
                    in_=hT8_v[1, 2 * mm:2 * mm + 2].rearrange("s p c -> p s c"))
            if any_mixed:
                nc.gpsimd.dma_start(out=mb_sb, in_=mb[:, :])
            for mm in range(4):
                nc.gpsimd.dma_start(
                    out=wot_sb[:, 2 * mm:2 * mm + 2, :],
                    in_=woT_v[2 * mm:2 * mm + 2].rearrange("s p c -> p s c"))
            nc.gpsimd.dma_start(out=gam_sb, in_=bass.AP(tensor=gamma, offset=0, ap=[[0, 128], [1, DM]]))
            nc.gpsimd.dma_start(out=bet_sb, in_=bass.AP(tensor=beta, offset=0, ap=[[0, 128], [1, DM]]))

            CH = HPC * DH  # 128 channels per section

            def qk_group(b, ct, dest, p):
                """One QKV projection group: 4 DR matmuls + copy to bf16."""
                htb = hts[b]
                ps = ps_pool.tile([128, 1024], F32, name="ps", tag="ps")
                for mm in range(4):
                    nc.tensor.matmul(
                        ps[:, 0:512],
                        lhsT=wt_sb[:, 2 * mm:2 * mm + 2, ct * CH:(ct + 1) * CH],
                        rhs=htb[:, 2 * mm:2 * mm + 2, p * 512:(p + 1) * 512],
                        start=(mm == 0), stop=(mm == 3), perf_mode=DR,
                    )
                nc.vector.tensor_copy(dest[:, b, p * 512:(p + 1) * 512], ps[:, 0:512])

            def v_mm(b, p):
                """v projection panel into vT (bf16)."""
                htb = hts[b]
                ps = ps_pool.tile([128, 1024], F32, name="ps", tag="ps")
                for mm in range(4):
                    nc.tensor.matmul(
                        ps[:, 0:512],
                        lhsT=wt_sb[:, 2 * mm:2 * mm + 2, 2 * CH:3 * CH],
                        rhs=htb[:, 2 * mm:2 * mm + 2, p * 512:(p + 1) * 512],
                        start=(mm == 0), stop=(mm == 3), perf_mode=DR,
                    )
                nc.vector.tensor_copy(vT[:, b, p * 512:(p + 1) * 512], ps[:, 0:512])

            def v_tc(b):
                """All transposes + fp8 casts for batch b, after all v_mm(b,*):
                keeps the vector queue free of transpose-gated entries ahead of
                later vT copies."""
                for p in range(4):
                    for h in range(HPC):
                        vpk = small.tile([128, 4, DH], BF, name="vpk", tag="vpk", bufs=4)
                        nc.sync.dma_start(
                            out=vpk, in_=vT[h * DH:(h + 1) * DH, b, p * 512:(p + 1) * 512],
                            transpose=True)
                        nc.vector.tensor_copy(
                            v_sb[:, b, p * 4:p * 4 + 4, :].rearrange(
                                "c t (h d) -> c t h d", h=2)[:, :, h, 0:DH],
                            vpk)

            # filler queue: independent matmul groups woven into the attention
            # stream to keep the PE dense (HAM stays at K=8/8)
            fillers = deque()

            def emit_fillers(n):
                for _ in range(n):
                    if not fillers:
                        return
                    fillers.popleft()()

            qT4 = qT.rearrange("c b (q f) -> c b q f", f=4)
            vec5 = vecT.rearrange("c p j (k two) -> c p j k two", two=2)

            def attn_panel(b, p, fps=1):
                ups = [u_ps.tile([65, 512], F32, name=f"ups{h}", tag=f"ups{h}")
                       for h in range(HPC)]
                grp = groups[b]
                for gi, g in enumerate(grp):
                    e2t = epool.tile([128, 2, 1024], FP8, name="e2", tag="e2", bufs=3)
                    for slot, t in enumerate(g):
                        sc = ps_pool.tile([128, 1024], F32, name="sc", tag="ps")
                        for h in range(HPC):
                            nc.tensor.matmul(
                                sc[:, h * 512:(h + 1) * 512],
                                lhsT=kT[h * DH:(h + 1) * DH, b, t * 128:(t + 1) * 128],
                                rhs=qT4[h * DH:(h + 1) * DH, b, :, p],
                                start=True, stop=True,
                            )
                        if any_mixed:
                            bias = mb_sb[:, t * BATCH + b:t * BATCH + b + 1]
                        else:
                            bias = expb_sb[:, 0:1]
                        nc.scalar.activation(
                            out=e2t[:, slot, :], in_=sc,
                            func=mybir.ActivationFunctionType.Exp,
                            bias=bias, scale=1.0 / (np.sqrt(DH) * WS * WS),
                        )
                    first, last = (gi == 0), (gi == len(grp) - 1)
                    for h in range(HPC):
                        if len(g) == 2:
                            nc.tensor.matmul(
                                ups[h],
                                lhsT=v_sb[:, b, g[0]:g[0] + 2, :].rearrange(
                                    "c t (h d) -> c t h d", h=2)[:, :, h, 0:DH + 1],
                                rhs=e2t[:, :, h * 512:(h + 1) * 512],
                                start=first, stop=last, perf_mode=DR,
                                skip_group_check=True,
                            )
                        else:
                            nc.tensor.matmul(
                                ups[h],
                                lhsT=v_sb[:, b, g[0], 80 * h:80 * h + DH + 1],
                                rhs=e2t[:, 0, h * 512:(h + 1) * 512],
                                start=first, stop=last,
                                skip_group_check=True,
                            )
                    emit_fillers(fps)
                # Z rows -> DRAM, packed reciprocal, broadcast back, divide
                u_sbs = []
                for h in range(HPC):
                    u_sb = small.tile([65, 512], F32, name="u_sb", tag=f"u_sb{h}", bufs=2)
                    nc.vector.tensor_copy(u_sb, ups[h])
                    nc.gpsimd.dma_start(out=zb[b, h, p, :], in_=u_sb[64:65, :])
                    u_sbs.append(u_sb)
                zpk = small.tile([128, 8], F32, name="zpk", tag="zpk", bufs=2)
                nc.gpsimd.dma_start(
                    out=zpk,
                    in_=bass.AP(tensor=zb, offset=(b * HPC * NP + p) * 512,
                                ap=[[NP * 512, 2], [1, 512]]))
                nc.vector.reciprocal(zpk, zpk)
                nc.gpsimd.dma_start(
                    out=bass.AP(tensor=zr, offset=(b * HPC * NP + p) * 512,
                                ap=[[NP * 512, 2], [1, 512]]),
                    in_=zpk)
                for h in range(HPC):
                    zrep = small.tile([64, 512], F32, name="zrep", tag="zrep", bufs=2)
                    nc.gpsimd.dma_start(
                        out=zrep,
                        in_=bass.AP(tensor=zr, offset=((b * HPC + h) * NP + p) * 512,
                                    ap=[[0, 64], [1, 512]]))
                    nc.vector.tensor_mul(
                        out=vec5[h * DH:(h + 1) * DH, p, :, :, b],
                        in0=u_sbs[h][0:DH, :], in1=zrep,
                    )

            def a2a_chunk(p):
                nc.gpsimd.dma_start(
                    out=a2a_in[p].rearrange("j c r -> c j r"),
                    in_=vecT[:, p, :, :])
                nc.gpsimd.collective_compute(
                    "AllToAll", mybir.AluOpType.bypass,
                    replica_groups=[list(range(NC_))],
                    ins=[a2a_in[p, :]], outs=[a2a_out[p, :]],
                )

            # Wo/LN per row block; DMAs early, matmul groups as fillers later
            wo_state = {}

            def wo_dma(p):
                vt = small.tile([128, 8, 128], FP8, name="vt", tag="vt", bufs=2)
                nc.gpsimd.dma_start(out=vt, in_=a2a_out[p].rearrange("i c r -> c i r"))
                hr = big.tile([128, DM], F32, name="hr", tag="hr", bufs=2)
                nc.gpsimd.dma_start(
                    out=hr,
                    in_=bass.AP(tensor=hrows, offset=2 * p * DM,
                                ap=[[8 * DM, 64], [DM, 2], [1, DM]]))
                x = big.tile([128, DM], F32, name="x", tag="x", bufs=2)
                wo_state[p] = (vt, hr, x)

            def wo_mh(p, mh):
                vt, hr, x = wo_state[p]
                ps = ps_pool.tile([128, 1024], F32, name="ps", tag="ps")
                for c4 in range(4):
                    nc.tensor.matmul(
                        ps[:, 0:512],
                        lhsT=vt[:, 2 * c4:2 * c4 + 2, :],
                        rhs=wot_sb[:, 2 * c4:2 * c4 + 2, mh * 512:(mh + 1) * 512],
                        start=(c4 == 0), stop=(c4 == 3), perf_mode=DR,
                    )
                nc.vector.tensor_add(
                    out=x[:, mh * 512:(mh + 1) * 512],
                    in0=ps[:, 0:512], in1=hr[:, mh * 512:(mh + 1) * 512],
                )

            def wo_ln(p):
                _, _, x = wo_state.pop(p)
                stats = small.tile([128, 2, 6], F32, name="stats", tag="stats")
                nc.vector.bn_stats(out=stats[:, 0, :], in_=x[:, 0:512])
                nc.vector.bn_stats(out=stats[:, 1, :], in_=x[:, 512:1024])
                mv = small.tile([128, 2], F32, name="mv", tag="mv")
                nc.vector.bn_aggr(out=mv, in_=stats)
                # rstd = (var+eps)^-0.5 = exp(-0.5*ln(var+eps)): Ln and Exp
                # share one ACT table, so no table reload between exps
                lnv = small.tile([128, 1], F32, name="lnv", tag="lnv")
                nc.scalar.activation(out=lnv, in_=mv[:, 1:2],
                                     func=mybir.ActivationFunctionType.Ln,
                                     bias=eps_sb, scale=1.0)
                rstd = small.tile([128, 1], F32, name="rstd", tag="rstd")
                nc.scalar.activation(out=rstd, in_=lnv,
                                     func=mybir.ActivationFunctionType.Exp,
                                     bias=zero_sb[:, 0:1], scale=-0.5)
                nc.vector.tensor_scalar(
                    out=x, in0=x,
                    scalar1=mv[:, 0:1], scalar2=rstd,
                    op0=mybir.AluOpType.subtract, op1=mybir.AluOpType.mult,
                )
                nc.vector.tensor_mul(out=x, in0=x, in1=gam_sb)
                nc.vector.tensor_add(out=x, in0=x, in1=bet_sb)
                nc.gpsimd.dma_start(
                    out=bass.AP(tensor=out, offset=2 * p * DM,
                                ap=[[8 * DM, 64], [DM, 2], [1, DM]]),
                    in_=x)

            # ---- emission ----
            # QKV b0 dense (warms the PE), b1 as fillers inside attention
            for ct, dest in ((1, kT), (0, qT)):
                for p in range(4):
                    qk_group(0, ct, dest, p)
            for p in range(4):
                v_mm(0, p)
            v_tc(0)
            for ct, dest in ((1, kT), (0, qT)):
                for p in range(4):
                    fillers.append(lambda b=1, ct=ct, dest=dest, p=p: qk_group(b, ct, dest, p))
            for p in range(4):
                fillers.append(lambda b=1, p=p: v_mm(b, p))
            fillers.append(lambda b=1: v_tc(b))

            for p in range(NP):
                for b in range(BATCH):
                    # all 12 QKV(b1) fillers must land inside (b0, p0) so
                    # (b1, p0)'s AV never precedes its v_group in the PE queue
                    attn_panel(b, p, fps=2 if (p == 0 and b == 0) else 1)
                a2a_chunk(p)
                if p >= 2:
                    wo_dma(p - 2)
                    fillers.append(lambda q=p - 2: wo_mh(q, 0))
                    fillers.append(lambda q=p - 2: wo_mh(q, 1))
                    fillers.append(lambda q=p - 2: wo_ln(q))
            while fillers:
                fillers.popleft()()
            for q in (NP - 2, NP - 1):
                wo_dma(q)
                wo_mh(q, 0)
                wo_mh(q, 1)
                wo_ln(q)

    _legalize_waits(nc)
    return nc


# ---------------------------------------------------------------------------
# host wrapper
# ---------------------------------------------------------------------------
_CACHE = {}
LAST_RESULT = None


def _get_nc(attn_mask: np.ndarray):
    masked_full = frozenset(
        (t, b) for t in range(NT) for b in range(BATCH)
        if attn_mask[t * 128:(t + 1) * 128, b].all()
    )
    any_mixed = any(
        attn_mask[t * 128:(t + 1) * 128, b].any() and (t, b) not in masked_full
        for t in range(NT) for b in range(BATCH)
    )
    key = (masked_full, any_mixed)
    if key not in _CACHE:
        _CACHE[key] = _build(masked_full, any_mixed)
    return _CACHE[key]


def _in_maps(h, attn_mask, Wqkv, Wo, gamma, beta):
    h = np.asarray(h, np.float32)
    attn_mask = np.asarray(attn_mask, bool)
    Wqkv = np.asarray(Wqkv, np.float32)
    Wo = np.asarray(Wo, np.float32)
    gamma = np.asarray(gamma, np.float32)
    beta = np.asarray(beta, np.float32)

    hT8 = np.ascontiguousarray(h.transpose(1, 2, 0)).astype(FP8NP)   # [B, DM, SEQ]
    h_flat = h.reshape(SEQ * BATCH, DM)
    woT = np.ascontiguousarray(Wo.T * WS).astype(FP8NP)              # [DM(ch), DM(m)]
    mb = np.zeros((128, NT * BATCH), np.float32)
    for t in range(NT):
        for b in range(BATCH):
            mb[:, t * BATCH + b] = np.where(
                attn_mask[t * 128:(t + 1) * 128, b], NEG, 0.0) + EXPB

    maps = []
    for c in range(NC_):
        h0, h1 = HPC * c, HPC * c + 1
        rows = []
        for sec in range(3):  # q, k, v
            for hh in (h0, h1):
                rows.append(Wqkv[sec * NH * DH + hh * DH: sec * NH * DH + (hh + 1) * DH])
        wTc = np.ascontiguousarray(np.concatenate(rows, 0).T * WS).astype(FP8NP)
        m = {
            "hT8": hT8,
            "wT": wTc,
            "woT": woT,
            "hrows": np.ascontiguousarray(h_flat[ROWS * c: ROWS * (c + 1)] * (WS * WS)),
            "gamma": gamma,
            "beta": beta,
            "mb": mb,
        }
        maps.append(m)
    return maps


def kernel(h, attn_mask, Wqkv, Wo, gamma, beta, _trace=False):
    global LAST_RESULT
    mask = np.asarray(attn_mask, bool)
    nc = _get_nc(mask)
    maps = _in_maps(h, attn_mask, Wqkv, Wo, gamma, beta)
    any_mixed = any(
        mask[t * 128:(t + 1) * 128, b].any() and not mask[t * 128:(t + 1) * 128, b].all()
        for t in range(NT) for b in range(BATCH)
    )
    if not any_mixed:
        for m in maps:
            m.pop("mb", None)
    res = run_bass_kernel_spmd(nc, maps, core_ids=list(range(NC_)), trace=_trace)
    LAST_RESULT = res
    full = np.concatenate([res.results[c]["out"] for c in range(NC_)], 0)
    out = full.reshape(SEQ, BATCH, DM)
    if _trace:
        return out, res.exec_time_ns
    return out


# revision 36
# speedup vs baseline: 1.1927x; 1.0226x over previous
"""Multi-head attention + residual + LayerNorm, tensor-parallel over heads
across 8 Trainium2 NeuronCores.

Reference computation (SEQ=2048, BATCH=2, D_MODEL=1024, H=16, D=64):
    qkv = h @ Wqkv.T ; per-(b,head) softmax((q k^T)/8, key-padding mask) @ v
    out = LayerNorm(h + concat_heads @ Wo.T) * gamma + beta

v4 strategy (2 heads per core):
- Projections (QKV, Wo) in fp8e4 DoubleRow; scores in bf16 with the two
  heads' K=64 matmuls row-tiled onto array halves (concurrent); AV as
  fp8 DoubleRow pairs of key tiles with a ones-column producing Z.
- Wqkv/Wo scaled x16 host-side; the resulting x256 on the attention
  branch is matched by hrows x256 and cancelled exactly by LayerNorm
  scale invariance.
- PE density: one shared 3-deep [128,1024] PSUM ring serves scores,
  QKV and Wo; QKV(b1) and Wo matmul groups are interleaved as fillers
  into the attention stream so the PE never idles long enough for the
  HAM clock gate to re-throttle to 1.2 GHz.
- Query panels interleaved mod 4: the vec AllToAll splits into 4
  chunks launched per finished panel; Wo+residual+LN per row block,
  two panels behind its chunk. Z reciprocals are computed on a
  [128,8] partition-packed tile.
"""
import sys

if "/opt/trn_rl_repo" not in sys.path:
    sys.path.insert(0, "/opt/trn_rl_repo")

from collections import deque

import numpy as np
import ml_dtypes

import bass_rust
import concourse.bass as bass
import concourse.mybir as mybir
import concourse.tile as tile
from concourse.bass_utils import run_bass_kernel_spmd

FP8NP = ml_dtypes.float8_e4m3fn
F32 = mybir.dt.float32
BF = mybir.dt.bfloat16
FP8 = mybir.dt.float8e4
DR = mybir.MatmulPerfMode.DoubleRow

SEQ, BATCH, DM = 2048, 2, 1024
NH, DH = 16, 64
NC_ = 8                      # cores
HPC = NH // NC_              # heads per core (2)
ROWS = SEQ * BATCH // NC_    # flat rows per core (512)
NT = SEQ // 128              # key tiles (16)
NP = 4                       # query panels (s mod 4 interleaved)
LN_EPS = 1e-5
NEG = -1e30
WS = 16.0                    # fp8 weight scale (power of two)
EXPB = -2.0                  # exp bias (cancels in U/Z), guards fp8 overflow


# ---------------------------------------------------------------------------
# walrus compat: this toolchain accepts at most ONE sync-wait per instruction.
# Split excess waits (and >1 updates on non-DMA instructions) onto adjacent
# same-engine NOPs after Tile scheduling.
# ---------------------------------------------------------------------------
_DMAISH = ("InstDMACopy", "InstDMATranspose", "DmaTranspose", "InstCollectiveCompute")


def _legalize_waits(nc: bass.Bass) -> int:
    n = 0
    for fn in nc.m.functions:
        for bb in fn.blocks:
            insts = bb.instructions
            i = 0
            while i < len(insts):
                inst = insts[i]
                si = inst.sync_info
                if si is None:
                    i += 1
                    continue
                waits = list(si.on_wait)
                updates = list(si.on_update)
                is_dma = any(k in type(inst).__name__ for k in _DMAISH)
                split_upd = (not is_dma) and len(updates) > 1
                if len(waits) <= 1 and not split_upd:
                    i += 1
                    continue
                keep_u = updates if not split_upd else updates[:1]
                extra_u = [] if not split_upd else updates[1:]
                eng = nc.engines[inst.engine]
                before = []
                for w in waits[1:]:
                    nop = eng.nop(nofuse=True).ins
                    _pop_last(nc, nop)
                    nop.sync_info = bass_rust.SyncInfo(on_wait=[w], on_update=[])
                    before.append(nop)
                after = []
                for u in extra_u:
                    nop = eng.nop(nofuse=True).ins
                    _pop_last(nc, nop)
                    nop.sync_info = bass_rust.SyncInfo(on_wait=[], on_update=[u])
                    after.append(nop)
                inst.sync_info = bass_rust.SyncInfo(on_wait=waits[:1], on_update=keep_u)
                insts[i:i + 1] = before + [inst] + after
                n += len(before) + len(after)
                i += len(before) + 1 + len(after)
    return n


def _pop_last(nc, inst):
    for fn in nc.m.functions:
        for bb in fn.blocks:
            lst = bb.instructions
            if lst and lst[-1] is inst:
                lst.pop()
                return
    for fn in nc.m.functions:
        for bb in fn.blocks:
            lst = bb.instructions
            for k in range(len(lst) - 1, -1, -1):
                if lst[k] is inst:
                    del lst[k]
                    return
    raise RuntimeError("fresh nop not found")


# ---------------------------------------------------------------------------
# kernel graph
# ---------------------------------------------------------------------------
def _build(masked_full, any_mixed):
    """masked_full: set of (t, b) key-tiles fully masked (skipped entirely).
    any_mixed: whether partially-masked tiles exist (bias from mb applied)."""
    nc = bass.Bass()

    hT8 = nc.declare_dram_parameter("hT8", [BATCH, DM, SEQ], FP8, isOutput=False)
    wT = nc.declare_dram_parameter("wT", [DM, 3 * HPC * DH], FP8, isOutput=False)
    woT = nc.declare_dram_parameter("woT", [DM, DM], FP8, isOutput=False)
    hrows = nc.declare_dram_parameter("hrows", [ROWS, DM], F32, isOutput=False)
    gamma = nc.declare_dram_parameter("gamma", [DM], F32, isOutput=False)
    beta = nc.declare_dram_parameter("beta", [DM], F32, isOutput=False)
    if any_mixed:
        mb = nc.declare_dram_parameter("mb", [128, NT * BATCH], F32, isOutput=False)
    out = nc.declare_dram_parameter("out", [ROWS, DM], F32, isOutput=True)

    a2a_in = nc.dram_tensor("a2a_in", [NP, BATCH, NC_, 128, 64], FP8)
    a2a_out = nc.dram_tensor("a2a_out", [NP, BATCH, NC_, 128, 64], FP8)
    zb = nc.dram_tensor("zb", [BATCH, HPC, NP, 512], F32)
    zr = nc.dram_tensor("zr", [BATCH, HPC, NP, 512], F32)

    # per batch: unmasked key tiles, grouped into consecutive DoubleRow pairs
    unmasked = {b: [t for t in range(NT) if (t, b) not in masked_full] for b in range(BATCH)}
    groups = {}
    for b in range(BATCH):
        um = unmasked[b]
        g, i = [], 0
        while i < len(um):
            if i + 1 < len(um) and um[i + 1] == um[i] + 1:
                g.append((um[i], um[i] + 1))
                i += 2
            else:
                g.append((um[i],))
                i += 1
        groups[b] = g

    with tile.TileContext(nc) as tc:
        with tc.tile_pool(name="big", bufs=1) as big, \
             tc.tile_pool(name="epool", bufs=3) as epool, \
             tc.tile_pool(name="small", bufs=4) as small, \
             tc.tile_pool(name="ps_pool", bufs=3, space="PSUM") as ps_pool, \
             tc.tile_pool(name="u_ps", bufs=1, space="PSUM") as u_ps:

            # ---- persistent SBUF tensors ----
            wt_sb = big.tile([128, 8, 3 * HPC * DH], FP8)     # Wqkv shard^T chunks
            wot_sb = big.tile([128, 8, DM], FP8)              # Wo^T chunks (full)
            qT = big.tile([128, BATCH, SEQ], BF)
            kT = big.tile([128, BATCH, SEQ], BF)
            vT = big.tile([128, BATCH, SEQ], BF)
            v_sb = big.tile([128, BATCH, NT, 160], FP8)       # [v_h0|1|pad|v_h1|1|pad]
            vecT = big.tile([128, NP, BATCH, NC_, 64], FP8)   # chunked vec^T
            eps_sb = big.tile([128, 1], F32)
            expb_sb = big.tile([128, 1], F32)
            gam_sb = big.tile([128, DM], F32)
            bet_sb = big.tile([128, DM], F32)
            if any_mixed:
                mb_sb = big.tile([128, NT * BATCH], F32)

            zero_sb = big.tile([128, 1], F32)
            nc.vector.memset(zero_sb, 0.0)
            nc.vector.memset(expb_sb, EXPB)
            nc.vector.memset(eps_sb, LN_EPS * (WS * WS) ** 2)
            nc.vector.memset(v_sb[:, :, :, 64:65], 1.0)
            nc.vector.memset(v_sb[:, :, :, 144:145], 1.0)

            # weights for first QKV pair, then h(b=0), then the rest
            wT_v = wT.rearrange("(m p) c -> m p c", p=128)     # [8, 128, 384]
            woT_v = woT.rearrange("(m p) c -> m p c", p=128)
            nc.gpsimd.dma_start(out=wt_sb[:, 0:2, :], in_=wT_v[0:2].rearrange("s p c -> p s c"))
            hts = []
            for b in range(BATCH):
                htb = big.tile([128, 8, SEQ], FP8, name=f"hts{b}", tag="hts", bufs=2)
                hts.append(htb)
            hT8_v = hT8.rearrange("b (m p) c -> b m p c", p=128)
            for mm in range(4):
                nc.gpsimd.dma_start(
                    out=hts[0][:, 2 * mm:2 * mm + 2, :],
                    in_=hT8_v[0, 2 * mm:2 * mm + 2].rearrange("s p c -> p s c"))
            for mm in range(1, 4):
                nc.gpsimd.dma_start(
                    out=wt_sb[:, 2 * mm:2 * mm + 2, :],
                    in_=wT_v[2 * mm:2 * mm + 2].rearrange("s p c -> p s c"))
            for mm in range(4):
                nc.gpsimd.dma_start(
                    out=hts[1][:, 2 * mm:2 * mm + 2, :],
                    in_=hT8_v[1, 2 * mm:2 * mm + 2].rearrange("s p c -> p s c"))
            if any_mixed:
                nc.gpsimd.dma_start(out=mb_sb, in_=mb[:, :])
            for mm in range(4):
                nc.gpsimd.dma_start(
                    out=wot_sb[:, 2 * mm:2 * mm + 2, :],
                    in_=woT_v[2 * mm:2 * mm + 2].rearrange("s p c -> p s c"))
            nc.gpsimd.dma_start(out=gam_sb, in_=bass.AP(tensor=gamma, offset=0, ap=[[0, 128], [1, DM]]))
            nc.gpsimd.dma_start(out=bet_sb, in_=bass.AP(tensor=beta, offset=0, ap=[[0, 128], [1, DM]]))

            CH = HPC * DH  # 128 channels per section

            def qk_group(b, ct, dest, p):
                """One QKV projection group: 4 DR matmuls + copy to bf16."""
                htb = hts[b]
                ps = ps_pool.tile([128, 1024], F32, name="ps", tag="ps")
                for mm in range(4):
                    nc.tensor.matmul(
                        ps[:, 0:512],
                        lhsT=wt_sb[:, 2 * mm:2 * mm + 2, ct * CH:(ct + 1) * CH],
                        rhs=htb[:, 2 * mm:2 * mm + 2, p * 512:(p + 1) * 512],
                        start=(mm == 0), stop=(mm == 3), perf_mode=DR,
                    )
                nc.vector.tensor_copy(dest[:, b, p * 512:(p + 1) * 512], ps[:, 0:512])

            def v_mm(b, p):
                """v projection panel into vT (bf16)."""
                htb = hts[b]
                ps = ps_pool.tile([128, 1024], F32, name="ps", tag="ps")
                for mm in range(4):
                    nc.tensor.matmul(
                        ps[:, 0:512],
                        lhsT=wt_sb[:, 2 * mm:2 * mm + 2, 2 * CH:3 * CH],
                        rhs=htb[:, 2 * mm:2 * mm + 2, p * 512:(p + 1) * 512],
                        start=(mm == 0), stop=(mm == 3), perf_mode=DR,
                    )
                nc.vector.tensor_copy(vT[:, b, p * 512:(p + 1) * 512], ps[:, 0:512])

            def v_tc(b):
                """All transposes + fp8 casts for batch b, after all v_mm(b,*):
                keeps the vector queue free of transpose-gated entries ahead of
                later vT copies."""
                for p in range(4):
                    for h in range(HPC):
                        vpk = small.tile([128, 4, DH], BF, name="vpk", tag="vpk", bufs=4)
                        nc.sync.dma_start(
                            out=vpk, in_=vT[h * DH:(h + 1) * DH, b, p * 512:(p + 1) * 512],
                            transpose=True)
                        nc.vector.tensor_copy(
                            v_sb[:, b, p * 4:p * 4 + 4, :].rearrange(
                                "c t (h d) -> c t h d", h=2)[:, :, h, 0:DH],
                            vpk)

            # filler queue: independent matmul groups woven into the attention
            # stream to keep the PE dense (HAM stays at K=8/8)
            fillers = deque()

            def emit_fillers(n):
                for _ in range(n):
                    if not fillers:
                        return
                    fillers.popleft()()

            qT4 = qT.rearrange("c b (q f) -> c b q f", f=4)

            def attn_panel(b, p, fps=1):
                ups = [u_ps.tile([65, 512], F32, name=f"ups{h}", tag=f"ups{h}")
                       for h in range(HPC)]
                grp = groups[b]
                for gi, g in enumerate(grp):
                    e2t = epool.tile([128, 2, 1024], FP8, name="e2", tag="e2", bufs=3)
                    for slot, t in enumerate(g):
                        sc = ps_pool.tile([128, 1024], F32, name="sc", tag="ps")
                        for h in range(HPC):
                            nc.tensor.matmul(
                                sc[:, h * 512:(h + 1) * 512],
                                lhsT=kT[h * DH:(h + 1) * DH, b, t * 128:(t + 1) * 128],
                                rhs=qT4[h * DH:(h + 1) * DH, b, :, p],
                                start=True, stop=True,
                            )
                        if any_mixed:
                            bias = mb_sb[:, t * BATCH + b:t * BATCH + b + 1]
                        else:
                            bias = expb_sb[:, 0:1]
                        nc.scalar.activation(
                            out=e2t[:, slot, :], in_=sc,
                            func=mybir.ActivationFunctionType.Exp,
                            bias=bias, scale=1.0 / (np.sqrt(DH) * WS * WS),
                        )
                    first, last = (gi == 0), (gi == len(grp) - 1)
                    for h in range(HPC):
                        if len(g) == 2:
                            nc.tensor.matmul(
                                ups[h],
                                lhsT=v_sb[:, b, g[0]:g[0] + 2, :].rearrange(
                                    "c t (h d) -> c t h d", h=2)[:, :, h, 0:DH + 1],
                                rhs=e2t[:, :, h * 512:(h + 1) * 512],
                                start=first, stop=last, perf_mode=DR,
                                skip_group_check=True,
                            )
                        else:
                            nc.tensor.matmul(
                                ups[h],
                                lhsT=v_sb[:, b, g[0], 80 * h:80 * h + DH + 1],
                                rhs=e2t[:, 0, h * 512:(h + 1) * 512],
                                start=first, stop=last,
                                skip_group_check=True,
                            )
                    emit_fillers(fps)
                # Z rows -> DRAM, packed reciprocal, broadcast back, divide.
                # All small DMAs ride the (otherwise idle) sync queue so the
                # gpsimd queue stays clear for collectives and Wo traffic.
                u_sbs = []
                for h in range(HPC):
                    u_sb = small.tile([65, 512], F32, name="u_sb", tag=f"u_sb{h}", bufs=2)
                    nc.vector.tensor_copy(u_sb, ups[h])
                    nc.sync.dma_start(out=zb[b, h, p, :], in_=u_sb[64:65, :])
                    u_sbs.append(u_sb)
                zpk = small.tile([128, 8], F32, name="zpk", tag="zpk", bufs=2)
                nc.sync.dma_start(
                    out=zpk,
                    in_=bass.AP(tensor=zb, offset=(b * HPC * NP + p) * 512,
                                ap=[[NP * 512, 2], [1, 512]]))
                nc.vector.reciprocal(zpk, zpk)
                nc.sync.dma_start(
                    out=bass.AP(tensor=zr, offset=(b * HPC * NP + p) * 512,
                                ap=[[NP * 512, 2], [1, 512]]),
                    in_=zpk)
                for h in range(HPC):
                    zrep = small.tile([64, 512], F32, name="zrep", tag="zrep", bufs=2)
                    nc.sync.dma_start(
                        out=zrep,
                        in_=bass.AP(tensor=zr, offset=((b * HPC + h) * NP + p) * 512,
                                    ap=[[0, 64], [1, 512]]))
                    nc.vector.tensor_mul(
                        out=vecT[h * DH:(h + 1) * DH, p, b, :, :],
                        in0=u_sbs[h][0:DH, :], in1=zrep,
                    )

            def a2a_chunk(p, b):
                nc.sync.dma_start(
                    out=a2a_in[p, b].rearrange("j c r -> c j r"),
                    in_=vecT[:, p, b, :, :])
                nc.gpsimd.collective_compute(
                    "AllToAll", mybir.AluOpType.bypass,
                    replica_groups=[list(range(NC_))],
                    ins=[a2a_in[p, b, :]], outs=[a2a_out[p, b, :]],
                )

            # Wo/LN per row block; DMAs early, matmul groups as fillers later
            wo_state = {}

            def wo_dma(p):
                # vt rows ordered (b, k): partition i = 64*b + k
                vt = small.tile([128, 8, 2, 64], FP8, name="vt", tag="vt", bufs=2)
                for b in range(BATCH):
                    nc.gpsimd.dma_start(
                        out=vt[:, :, b, :],
                        in_=a2a_out[p, b].rearrange("i c k -> c i k"))
                hr = big.tile([128, DM], F32, name="hr", tag="hr", bufs=2)
                nc.gpsimd.dma_start(
                    out=hr,
                    in_=bass.AP(tensor=hrows, offset=2 * p * DM,
                                ap=[[DM, 2], [8 * DM, 64], [1, DM]]))
                x = big.tile([128, DM], F32, name="x", tag="x", bufs=2)
                wo_state[p] = (vt, hr, x)

            def wo_mh(p, mh):
                vt, hr, x = wo_state[p]
                ps = ps_pool.tile([128, 1024], F32, name="ps", tag="ps")
                for c4 in range(4):
                    nc.tensor.matmul(
                        ps[:, 0:512],
                        lhsT=vt[:, 2 * c4:2 * c4 + 2, :, :],
                        rhs=wot_sb[:, 2 * c4:2 * c4 + 2, mh * 512:(mh + 1) * 512],
                        start=(c4 == 0), stop=(c4 == 3), perf_mode=DR,
                    )
                nc.vector.tensor_add(
                    out=x[:, mh * 512:(mh + 1) * 512],
                    in0=ps[:, 0:512], in1=hr[:, mh * 512:(mh + 1) * 512],
                )

            def wo_ln(p):
                _, _, x = wo_state.pop(p)
                stats = small.tile([128, 2, 6], F32, name="stats", tag="stats")
                nc.vector.bn_stats(out=stats[:, 0, :], in_=x[:, 0:512])
                nc.vector.bn_stats(out=stats[:, 1, :], in_=x[:, 512:1024])
                mv = small.tile([128, 2], F32, name="mv", tag="mv")
                nc.vector.bn_aggr(out=mv, in_=stats)
                # rstd = (var+eps)^-0.5 = exp(-0.5*ln(var+eps)): Ln and Exp
                # share one ACT table, so no table reload between exps
                lnv = small.tile([128, 1], F32, name="lnv", tag="lnv")
                nc.scalar.activation(out=lnv, in_=mv[:, 1:2],
                                     func=mybir.ActivationFunctionType.Ln,
                                     bias=eps_sb, scale=1.0)
                rstd = small.tile([128, 1], F32, name="rstd", tag="rstd")
                nc.scalar.activation(out=rstd, in_=lnv,
                                     func=mybir.ActivationFunctionType.Exp,
                                     bias=zero_sb[:, 0:1], scale=-0.5)
                nc.vector.tensor_scalar(
                    out=x, in0=x,
                    scalar1=mv[:, 0:1], scalar2=rstd,
                    op0=mybir.AluOpType.subtract, op1=mybir.AluOpType.mult,
                )
                nc.vector.tensor_mul(out=x, in0=x, in1=gam_sb)
                nc.vector.tensor_add(out=x, in0=x, in1=bet_sb)
                nc.gpsimd.dma_start(
                    out=bass.AP(tensor=out, offset=2 * p * DM,
                                ap=[[DM, 2], [8 * DM, 64], [1, DM]]),
                    in_=x)

            # ---- emission ----
            # Dense QKV for both batches up front: back-to-back matmuls warm
            # the HAM clock gate; attention is then ACT(exp)-paced with Wo
            # blocks woven in as fillers.
            for b in range(BATCH):
                for ct, dest in ((1, kT), (0, qT)):
                    for p in range(4):
                        qk_group(b, ct, dest, p)
            for b in range(BATCH):
                for p in range(4):
                    v_mm(b, p)
            for b in range(BATCH):
                v_tc(b)

            for p in range(NP):
                for b in range(BATCH):
                    attn_panel(b, p)
                    a2a_chunk(p, b)
                if p >= 2:
                    # chunk (p-2, b1) landed a full panel ago
                    wo_dma(p - 2)
                    fillers.append(lambda q=p - 2: wo_mh(q, 0))
                    fillers.append(lambda q=p - 2: wo_mh(q, 1))
                    fillers.append(lambda q=p - 2: wo_ln(q))
            while fillers:
                fillers.popleft()()
            for q in (NP - 2, NP - 1):
                wo_dma(q)
                wo_mh(q, 0)
                wo_mh(q, 1)
                wo_ln(q)

    _legalize_waits(nc)
    return nc


# ---------------------------------------------------------------------------
# host wrapper
# ---------------------------------------------------------------------------
_CACHE = {}
LAST_RESULT = None


def _get_nc(attn_mask: np.ndarray):
    masked_full = frozenset(
        (t, b) for t in range(NT) for b in range(BATCH)
        if attn_mask[t * 128:(t + 1) * 128, b].all()
    )
    any_mixed = any(
        attn_mask[t * 128:(t + 1) * 128, b].any() and (t, b) not in masked_full
        for t in range(NT) for b in range(BATCH)
    )
    key = (masked_full, any_mixed)
    if key not in _CACHE:
        _CACHE[key] = _build(masked_full, any_mixed)
    return _CACHE[key]


def _in_maps(h, attn_mask, Wqkv, Wo, gamma, beta):
    h = np.asarray(h, np.float32)
    attn_mask = np.asarray(attn_mask, bool)
    Wqkv = np.asarray(Wqkv, np.float32)
    Wo = np.asarray(Wo, np.float32)
    gamma = np.asarray(gamma, np.float32)
    beta = np.asarray(beta, np.float32)

    hT8 = np.ascontiguousarray(h.transpose(1, 2, 0)).astype(FP8NP)   # [B, DM, SEQ]
    h_flat = h.reshape(SEQ * BATCH, DM)
    woT = np.ascontiguousarray(Wo.T * WS).astype(FP8NP)              # [DM(ch), DM(m)]
    mb = np.zeros((128, NT * BATCH), np.float32)
    for t in range(NT):
        for b in range(BATCH):
            mb[:, t * BATCH + b] = np.where(
                attn_mask[t * 128:(t + 1) * 128, b], NEG, 0.0) + EXPB

    maps = []
    for c in range(NC_):
        h0, h1 = HPC * c, HPC * c + 1
        rows = []
        for sec in range(3):  # q, k, v
            for hh in (h0, h1):
                rows.append(Wqkv[sec * NH * DH + hh * DH: sec * NH * DH + (hh + 1) * DH])
        wTc = np.ascontiguousarray(np.concatenate(rows, 0).T * WS).astype(FP8NP)
        m = {
            "hT8": hT8,
            "wT": wTc,
            "woT": woT,
            "hrows": np.ascontiguousarray(h_flat[ROWS * c: ROWS * (c + 1)] * (WS * WS)),
            "gamma": gamma,
            "beta": beta,
            "mb": mb,
        }
        maps.append(m)
    return maps


def kernel(h, attn_mask, Wqkv, Wo, gamma, beta, _trace=False):
    global LAST_RESULT
    mask = np.asarray(attn_mask, bool)
    nc = _get_nc(mask)
    maps = _in_maps(h, attn_mask, Wqkv, Wo, gamma, beta)
    any_mixed = any(
        mask[t * 128:(t + 1) * 128, b].any() and not mask[t * 128:(t + 1) * 128, b].all()
        for t in range(NT) for b in range(BATCH)
    )
    if not any_mixed:
        for m in maps:
            m.pop("mb", None)
    res = run_bass_kernel_spmd(nc, maps, core_ids=list(range(NC_)), trace=_trace)
    LAST_RESULT = res
    full = np.concatenate([res.results[c]["out"] for c in range(NC_)], 0)
    out = full.reshape(SEQ, BATCH, DM)
    if _trace:
        return out, res.exec_time_ns
    return out
